# revision 1
# baseline (speedup 1.0000x reference)
"""GCN (2-layer + mean-pool + MLP) on 8 Trainium2 NeuronCores.

Strategy (dst-sharded message passing, matmul-based segment sum):
  - Nodes are tiled into 128-row tiles; each core owns 98 tiles (12544 nodes).
  - Edges are bucketed by (dst tile, src chunk) with a fixed per-bucket
    capacity C=256 (two 128-edge windows); bucket overflow beyond C spills
    into one shared overflow window per (supergroup, chunk), whose wsel
    passes target each tile of the supergroup in turn.  This keeps the
    instruction stream identical on every core (SPMD) while cutting gather
    descriptors ~25% vs padding every bucket to the max count.  Self loops
    get a dedicated window per tile whose source rows are contiguous in the
    local shard, so they are fetched with plain strided DMA, not dma_gather.
  - Aggregation S^T[f, d] = sum_e w_e * X[src_e, f] is computed on the PE:
    per 128-edge window, a weighted selection matrix Wsel[e, d] is built on
    the vector engine (iota==dstloc)*w via one dual-op tensor_scalar, and
    matmul(lhsT=G_window[e, f], rhs=Wsel) accumulates into the dst tile's
    PSUM region.  G windows are fetched with dma_gather (int16 indices,
    4 source chunks of 25088 rows each to satisfy the int16 range).
  - norm w_e = deg^-1/2[src] * deg^-1/2[dst] is folded into Wsel.
  - h1 is stored and exchanged in fp8e4m3 (halves the AllGather, the
    dominant serial cost; the quantization noise washes out in the
    mean-pool).  Layer 2 gathers fp8 rows and runs its window matmuls in
    fp8 x fp8 -> f32 PSUM.
  - PSUM->SBUF copies run on the Activation engine (Copy/Relu activations)
    to keep the vector engine free for Wsel builds; the mean-pool 1/cnt is
    folded into the h2 PSUM->SBUF copy as a per-partition activation scale.
  - Per-graph mean-pooling is a second selection matmul (Bsel =
    (iota512==batch)) accumulated over all tiles; pooled sums are AllReduced
    and the small MLP runs replicated.
"""

import os
import numpy as np
import ml_dtypes

from concourse import bass, mybir, tile, bacc
from concourse.bass_utils import run_bass_kernel_spmd

BF16 = mybir.dt.bfloat16
F32 = mybir.dt.float32
F8 = mybir.dt.float8e4
I16 = mybir.dt.int16
AF = mybir.ActivationFunctionType
ALU = mybir.AluOpType
NPF8 = mybir.dt.np(F8)

# problem sizes (hardcoded; see module docstring)
N, E, G = 100000, 800000, 512
DIN, DH, DMLP = 128, 256, 512
NCORES = 8
P = 128
TPC = 98                     # tiles per core (uniform; tail cores padded)
NT = NCORES * TPC            # 784 virtual tiles
NROWS = NT * P               # 100352 table rows (row == node id)
NCHUNK = 4
CHUNK = 25088                # source chunk rows (< 32768 for int16 idx)
SGS = [4] * 24 + [2]         # tiles per super-group, sum == TPC
SG_OF = []                   # tile-in-core -> (sg index, pos in sg)
for _si, _s in enumerate(SGS):
    for _j in range(_s):
        SG_OF.append((_si, _j))


def _prep(X, edge_index, batch, W1, b1, W2, b2, Wm1, bm1, Wm2, bm2):
    # self loops are handled by a dedicated per-sg bucket loaded contiguously
    # from the core-local shard (Xself / h1shard), so the random buckets stay
    # small.  Two edge streams per core: layer 1 gathers X, layer 2 gathers
    # the AllGathered h1 table (both in global row order).
    src = np.asarray(edge_index[0]).astype(np.int64)
    dst = np.asarray(edge_index[1]).astype(np.int64)
    batch = np.asarray(batch).astype(np.int64)
    SHARD = TPC * P

    deg = np.bincount(dst, minlength=N).astype(np.float32) + 1.0  # + self loop
    p = deg ** -0.5
    w = (p[src] * p[dst]).astype(np.float32)

    tile_id = (dst >> 7).astype(np.int64)        # 0..781
    core_of = tile_id // TPC

    # per-layer chunk ids and int16 index values
    chunk0 = src // CHUNK
    idxv0 = (src - chunk0 * CHUNK).astype(np.int16)
    rel_row = src                                # global row order (AllGather)
    chunk1 = rel_row // CHUNK
    idxv1 = (rel_row - chunk1 * CHUNK).astype(np.int16)

    C = 256                                      # per-tile bucket capacity
    OVF = 128                                    # shared overflow per (sg,k)
    WPT = C // P

    t_in_core = tile_id % TPC
    sg_idx_l = np.array([SG_OF[t][0] for t in range(TPC)])
    ti_sg_l = np.array([SG_OF[t][1] for t in range(TPC)])
    sg_sizes = np.array(SGS)
    # gather slots per sg: NCHUNK * (S*C + OVF); self loaded separately
    sg_slot_span = sg_sizes * NCHUNK * C + NCHUNK * OVF
    sg_slot_off = np.zeros(len(SGS) + 1, np.int64)
    np.cumsum(sg_slot_span, out=sg_slot_off[1:])
    SLOTS = int(sg_slot_off[-1])                 # gather slots per core
    # dl/w columns per sg: NCHUNK*(S*WPT regular + 1 ovf) + S self
    sg_col_span = sg_sizes * (NCHUNK * WPT + 1) + NCHUNK
    sg_col_off = np.zeros(len(SGS) + 1, np.int64)
    np.cumsum(sg_col_span, out=sg_col_off[1:])
    NW = int(sg_col_off[-1])                     # wsel columns per core

    sgi = sg_idx_l[t_in_core]
    tis = ti_sg_l[t_in_core]
    Ssz = sg_sizes[sgi]                          # sg size per edge

    def build_stream(chunk_id, idxv):
        key = tile_id * NCHUNK + chunk_id
        cnt = np.bincount(key, minlength=NT * NCHUNK)
        order = np.argsort(key, kind="stable")
        k_sorted = key[order]
        starts = np.zeros(NT * NCHUNK + 1, np.int64)
        np.cumsum(cnt, out=starts[1:])
        rank = np.empty(len(order), np.int64)
        rank[order] = np.arange(len(order)) - starts[k_sorted]

        reg = rank < C
        # overflow rank within (core, sg, chunk)
        okey = (core_of * len(SGS) + sgi) * NCHUNK + chunk_id
        osel = ~reg
        oorder = np.argsort(okey[osel], kind="stable")
        ocnt = np.bincount(okey[osel], minlength=NCORES * len(SGS) * NCHUNK)
        assert ocnt.max() <= OVF, f"overflow {ocnt.max()} > {OVF}"
        ostarts = np.zeros(NCORES * len(SGS) * NCHUNK + 1, np.int64)
        np.cumsum(ocnt, out=ostarts[1:])
        ovr = np.empty(osel.sum(), np.int64)
        ovr[oorder] = np.arange(osel.sum()) - ostarts[np.sort(okey[osel])]
        ovf_rank = np.zeros(len(src), np.int64)
        ovf_rank[osel] = ovr

        # gather slot of each edge
        kbase = sg_slot_off[sgi] + chunk_id * (Ssz * C + OVF)
        slot = np.where(
            reg,
            kbase + tis * C + rank,
            kbase + Ssz * C + ovf_rank,
        )
        # wsel column of each edge (one shared ovf column per (sg,k))
        cbase = sg_col_off[sgi] + chunk_id * (Ssz * WPT + 1)
        colid = np.where(
            reg,
            cbase + tis * WPT + rank // P,
            cbase + Ssz * WPT,
        )
        # position within the window
        pwin = np.where(reg, rank % P, ovf_rank)

        idx_flat = np.zeros((NCORES, SLOTS), np.int16)
        dl_flat = np.full((NCORES, NW, P), -1.0, np.float32)
        w_flat = np.zeros((NCORES, NW, P), np.float32)
        idx_flat[core_of, slot] = idxv
        sg_t0 = np.concatenate([[0], np.cumsum(sg_sizes)])[sgi]
        sg_base = (core_of * TPC + sg_t0) * P
        dl_flat[core_of, colid, pwin] = np.where(
            reg, dst - tile_id * P, dst - sg_base
        ).astype(np.float32)
        w_flat[core_of, colid, pwin] = w
        # self-loop entries: node n (real) -> self column of its sg
        nodes = np.arange(N)
        tl = nodes >> 7
        tc = tl % TPC
        co = tl // TPC
        sgi_n = sg_idx_l[tc]
        scol = (
            sg_col_off[sgi_n]
            + NCHUNK * (sg_sizes[sgi_n] * WPT + 1)
            + ti_sg_l[tc]
        )
        dl_flat[co, scol, nodes % P] = (nodes % P).astype(np.float32)
        w_flat[co, scol, nodes % P] = (p[nodes] ** 2).astype(np.float32)
        return idx_flat, dl_flat, w_flat

    idx_flat0, dl_flat0, w_flat0 = build_stream(chunk0, idxv0)
    idx_flat1, dl_flat1, w_flat1 = build_stream(chunk1, idxv1)

    mm_start = mm_stop = None                    # flags derived inline

    in_maps = []
    tbl = np.zeros((NROWS, DIN), ml_dtypes.bfloat16)
    tbl[:N] = np.asarray(X).astype(ml_dtypes.bfloat16)

    cnts_g = np.bincount(batch, minlength=G).astype(np.float32)
    invc = (1.0 / np.maximum(cnts_g, 1.0)).astype(np.float32)

    common = dict(
        tbl=tbl,
        iota128=np.tile(np.arange(P, dtype=ml_dtypes.bfloat16)[None, :], (P, 1)),
        iota512=np.tile(np.arange(G, dtype=np.float32)[None, :], (P, 1)),
        W1=np.asarray(W1).astype(ml_dtypes.bfloat16),
        b1=np.asarray(b1).astype(ml_dtypes.bfloat16)[None, :],
        W2lo=np.asarray(W2[:128]).astype(ml_dtypes.bfloat16),
        W2hi=np.asarray(W2[128:]).astype(ml_dtypes.bfloat16),
        b2=np.asarray(b2).astype(ml_dtypes.bfloat16)[None, :],
        Wm1lo=np.asarray(Wm1[:128]).astype(ml_dtypes.bfloat16),
        Wm1hi=np.asarray(Wm1[128:]).astype(ml_dtypes.bfloat16),
        # layer-2 bias folded through the linear mean-pool: bm1 += b2 @ Wm1
        bm1=(
            np.asarray(bm1).astype(np.float32)
            + np.asarray(b2).astype(np.float32) @ np.asarray(Wm1).astype(np.float32)
        ).reshape(4, 128).T.copy(),
        Wm2=np.asarray(Wm2).astype(ml_dtypes.bfloat16).reshape(4, 128).T.copy(),
    )
    bm2_f = float(np.asarray(bm2).reshape(-1)[0])
    common["bm2c"] = np.full((P, 1), bm2_f, np.float32)

    IDXBLOB = SLOTS                          # gather slots per core

    def wrap_idx(idx_flat, c):
        idx_c = idx_flat[c]
        # per-(sg,k) [16, L/16] wrap, replicated to 128 partitions
        idxw = np.empty((16, IDXBLOB // 16), np.int16)
        off = 0
        for si, S in enumerate(SGS):
            for L in [S * C + OVF] * NCHUNK:
                a = idx_c[off : off + L].reshape(L // 16, 16).T
                idxw[:, off // 16 : (off + L) // 16] = a
                off += L
        return np.tile(idxw, (8, 1))

    for c in range(NCORES):
        idxw0 = wrap_idx(idx_flat0, c)
        idxw1 = wrap_idx(idx_flat1, c)
        xself = np.zeros((TPC * P, DIN), ml_dtypes.bfloat16)
        lo, hi = c * TPC * P, min((c + 1) * TPC * P, N)
        xself[: hi - lo] = np.asarray(X[lo:hi]).astype(ml_dtypes.bfloat16)

        dl0_c = dl_flat0[c].T.copy()
        w0_c = w_flat0[c].T.copy()
        dl1_c = dl_flat1[c].T.copy()
        w1_c = w_flat1[c].T.copy()

        # pooling metadata per tile
        n0 = c * TPC * P
        nodes = np.arange(n0, n0 + TPC * P)
        valid = nodes < N
        bl = np.where(valid, batch[np.minimum(nodes, N - 1)], -1).astype(np.float32)
        iv = np.where(valid, invc[batch[np.minimum(nodes, N - 1)]], 0.0).astype(
            np.float32
        )
        in_maps.append(
            dict(
                common,
                idx0=idxw0,
                idx1=idxw1,
                dloc0=dl0_c,
                wgt0=w0_c,
                dloc1=dl1_c,
                wgt1=w1_c,
                xself=xself,
                bloc=bl.reshape(TPC, P).T.copy(),
                binv=iv.reshape(TPC, P).T.copy(),
            )
        )
    return in_maps, C, WPT, NW, mm_start, mm_stop, bm2_f


def _build(C, WPT, NW, mm_start, mm_stop, bm2_f):
    nc = bacc.Bacc(
        "TRN2",
        target_bir_lowering=False,
        debug=False,
        num_devices=NCORES,
        dynamic_dma_scratch_size=32768,
    )

    NSG = len(SGS)
    OVF = 128
    IDXBLOB = NCHUNK * (C * TPC + OVF * NSG)
    tbl = nc.dram_tensor("tbl", [NROWS, DIN], BF16, kind="ExternalInput")
    xself = nc.dram_tensor("xself", [TPC * P, DIN], BF16, kind="ExternalInput")
    idx0 = nc.dram_tensor("idx0", [P, IDXBLOB // 16], I16, kind="ExternalInput")
    idx1 = nc.dram_tensor("idx1", [P, IDXBLOB // 16], I16, kind="ExternalInput")
    dloc0 = nc.dram_tensor("dloc0", [P, NW], F32, kind="ExternalInput")
    wgt0 = nc.dram_tensor("wgt0", [P, NW], F32, kind="ExternalInput")
    dloc1 = nc.dram_tensor("dloc1", [P, NW], F32, kind="ExternalInput")
    wgt1 = nc.dram_tensor("wgt1", [P, NW], F32, kind="ExternalInput")
    bloc = nc.dram_tensor("bloc", [P, TPC], F32, kind="ExternalInput")
    binv = nc.dram_tensor("binv", [P, TPC], F32, kind="ExternalInput")
    iota128 = nc.dram_tensor("iota128", [P, P], BF16, kind="ExternalInput")
    iota512 = nc.dram_tensor("iota512", [P, G], F32, kind="ExternalInput")
    W1 = nc.dram_tensor("W1", [DIN, DH], BF16, kind="ExternalInput")
    b1 = nc.dram_tensor("b1", [1, DH], BF16, kind="ExternalInput")
    W2lo = nc.dram_tensor("W2lo", [P, DH], BF16, kind="ExternalInput")
    W2hi = nc.dram_tensor("W2hi", [P, DH], BF16, kind="ExternalInput")
    b2 = nc.dram_tensor("b2", [1, DH], BF16, kind="ExternalInput")
    Wm1lo = nc.dram_tensor("Wm1lo", [P, DMLP], BF16, kind="ExternalInput")
    Wm1hi = nc.dram_tensor("Wm1hi", [P, DMLP], BF16, kind="ExternalInput")
    bm1 = nc.dram_tensor("bm1", [P, 4], F32, kind="ExternalInput")
    Wm2 = nc.dram_tensor("Wm2", [P, 4], BF16, kind="ExternalInput")
    bm2c = nc.dram_tensor("bm2c", [P, 1], F32, kind="ExternalInput")
    out = nc.dram_tensor("out", [1, G], F32, kind="ExternalOutput")

    NIDX16 = IDXBLOB // 16
    GC1 = (4 * 2 + 1) * DIN                      # L1 gather buffer elems
    GC2 = (4 * 2 + 1) * DH                       # L2 gather buffer elems

    with tile.TileContext(nc) as tc:
        with (
            tc.tile_pool(name="const", bufs=1) as cp,
            tc.tile_pool(name="gbuf", bufs=10) as gp,
            tc.tile_pool(name="wselp", bufs=24) as wp,
            tc.tile_pool(name="stage", bufs=6) as sp,
            tc.tile_pool(name="bselp", bufs=8) as bp,
            tc.tile_pool(name="selpsum", bufs=4, space="PSUM") as pselp,
            tc.tile_pool(name="hpsum", bufs=2, space="PSUM") as php,
            tc.tile_pool(name="poolpsum", bufs=1, space="PSUM") as ppl,
            tc.tile_pool(name="dram", bufs=1, space="DRAM") as dp,
        ):
            # ---- resident constants / metadata ----
            # one idx tile, reloaded with the layer-2 stream between layers
            idx_t = cp.tile([P, NIDX16], I16)
            nc.sync.dma_start(idx_t[:], idx0[:, :])
            dl_ts = [cp.tile([P, NW], F32, name=f"dl{l}") for l in range(2)]
            w_ts = [cp.tile([P, NW], F32, name=f"w{l}") for l in range(2)]
            nc.sync.dma_start(dl_ts[0][:], dloc0[:, :])
            nc.sync.dma_start(w_ts[0][:], wgt0[:, :])
            bl_t = cp.tile([P, TPC], F32)
            nc.sync.dma_start(bl_t[:], bloc[:, :])
            bi_t = cp.tile([P, TPC], F32)
            nc.sync.dma_start(bi_t[:], binv[:, :])
            io128 = cp.tile([P, P], BF16)
            nc.sync.dma_start(io128[:], iota128[:, :])
            io512 = cp.tile([P, G], F32)
            nc.sync.dma_start(io512[:], iota512[:, :])
            W1_t = cp.tile([DIN, DH], BF16)
            nc.sync.dma_start(W1_t[:], W1[:, :])
            b1_t = cp.tile([P, DH], BF16)
            nc.sync.dma_start(b1_t[:1, :], b1[:, :])
            W2lo_t = cp.tile([P, DH], BF16)
            nc.sync.dma_start(W2lo_t[:], W2lo[:, :])
            W2hi_t = cp.tile([P, DH], BF16)
            nc.sync.dma_start(W2hi_t[:], W2hi[:, :])
            b2_t = cp.tile([P, DH], BF16)
            nc.sync.dma_start(b2_t[:1, :], b2[:, :])
            ones_t = cp.tile([P, P], BF16)
            nc.vector.memset(ones_t[:], 1.0)

            h1shard = dp.tile([TPC * P, DH], F8)
            # Shared addr space: HBM-HBM AllGather writes peer-visible
            # memory directly (the runtime warns Local is slower).
            h1full = nc.dram_tensor(
                "h1full_sh", [NROWS, DH], F8, kind="Internal",
                addr_space="Shared",
            )
            # ================= layer phase =================
            def layer(lidx):
                """lidx 0: gather X(128) -> h1; lidx 1: gather h1(256) -> h2+pool."""
                feat = DIN if lidx == 0 else DH
                gdt = BF16 if lidx == 0 else F8
                nfh = feat // P                      # feature halves
                src_tbl = tbl if lidx == 0 else h1full
                self_tbl = xself if lidx == 0 else h1shard
                dl_t = dl_ts[lidx]
                w_t = w_ts[lidx]
                tglob = 0
                off16 = 0
                cbase = 0                        # column base of current sg
                if lidx == 1:
                    # swap in the layer-2 gather indices / edge metadata
                    nc.sync.dma_start(idx_t[:], idx1[:, :])
                    nc.sync.dma_start(dl_ts[1][:], dloc1[:, :])
                    nc.sync.dma_start(w_ts[1][:], wgt1[:, :])
                    pl_lo = ppl.tile([P, G], F32, space="PSUM")
                    pl_hi = ppl.tile([P, G], F32, space="PSUM")
                for si, S in enumerate(SGS):
                    # PSUM for this sg: one accumulation region [P, P] per
                    # (tile, fhalf); regions packed 4-per-bank, one pool tile
                    # per 2KB bank (start=True zeroes a whole bank).
                    nbank = (S * nfh + 3) // 4
                    selbk = [
                        pselp.tile(
                            [P, 512], F32, space="PSUM", tag="selp",
                            name=f"selbk_{lidx}_{si}_{b}",
                        )
                        for b in range(nbank)
                    ]
                    for k in range(NCHUNK + 1):      # 4 chunk buckets + self
                        is_self = k == NCHUNK
                        L = S * P if is_self else S * C + OVF
                        nwin = L // P
                        g_t = gp.tile(
                            [P, GC1 if lidx == 0 else GC2], gdt, tag=f"gbuf{lidx}"
                        )
                        gv = g_t[:, : nwin * feat].rearrange(
                            "p (n f) -> p n f", f=feat
                        )
                        if is_self:
                            # self rows are contiguous in the local shard:
                            # plain strided DMA instead of dma_gather.  L1
                            # issues them from the scalar queue to keep SP
                            # free for const loads / h1 writes.
                            for ti in range(S):
                                t = tglob + ti
                                eng = (
                                    (nc.scalar if t % 2 else nc.sync)
                                    if lidx == 0
                                    else nc.sync
                                )
                                eng.dma_start(
                                    gv[:, ti, :],
                                    self_tbl[t * P : (t + 1) * P, :],
                                )
                        else:
                            # <=8 windows (1024 idxs) per dma_gather call:
                            # larger calls exceed the SWDGE ring sizing the
                            # terminal ucode assumes and crash the worker.
                            src_ap = src_tbl[k * CHUNK : (k + 1) * CHUNK, :]
                            for g0 in range(0, nwin, 8):
                                gn = min(8, nwin - g0)
                                nc.gpsimd.dma_gather(
                                    out_ap=gv[:, g0 : g0 + gn, :],
                                    in_ap=src_ap,
                                    idxs_ap=idx_t[
                                        :, off16 + g0 * 8 : off16 + (g0 + gn) * 8
                                    ],
                                    num_idxs=gn * P,
                                    num_idxs_reg=gn * P,
                                    elem_size=feat,
                                )
                            off16 += L // 16
                        # window index per (tile, wi): regular 2 per tile,
                        # plus one shared overflow window (last) per chunk.
                        # The ovf wsel is built ONCE per (sg,k) spanning the
                        # whole sg (dloc = dst - sg_base); each tile's matmul
                        # consumes its 128-column slice, which is exactly the
                        # per-tile one-hot.
                        wpb = 1 if is_self else WPT + 1
                        for ti in range(S):
                            for wi in range(wpb):
                                is_ovf = (not is_self) and wi == WPT
                                if is_self:
                                    ccol = (
                                        cbase + NCHUNK * (S * WPT + 1) + ti
                                    )
                                elif is_ovf:
                                    ccol = (
                                        cbase + k * (S * WPT + 1) + S * WPT
                                    )
                                else:
                                    ccol = (
                                        cbase + k * (S * WPT + 1)
                                        + ti * WPT + wi
                                    )
                                if is_ovf:
                                    if ti == 0:
                                        wselo = wp.tile(
                                            [P, 4 * P], gdt,
                                            tag=f"wselo{lidx}",
                                        )
                                        nc.vector.tensor_scalar(
                                            out=wselo[:, : S * P],
                                            in0=io512[:, : S * P],
                                            scalar1=dl_t[:, ccol : ccol + 1],
                                            scalar2=w_t[:, ccol : ccol + 1],
                                            op0=ALU.is_equal,
                                            op1=ALU.mult,
                                        )
                                    wsel_ap = wselo[:, ti * P : (ti + 1) * P]
                                else:
                                    wsel = wp.tile(
                                        [P, P], gdt, tag=f"wsel{lidx}"
                                    )
                                    # L1 self wsels alternate onto gpsimd:
                                    # Pool has slack in L1 while DVE binds.
                                    weng = (
                                        nc.gpsimd
                                        if is_self and lidx == 0
                                        else nc.vector
                                    )
                                    weng.tensor_scalar(
                                        out=wsel[:],
                                        in0=io128[:],
                                        scalar1=dl_t[:, ccol : ccol + 1],
                                        scalar2=w_t[:, ccol : ccol + 1],
                                        op0=ALU.is_equal,
                                        op1=ALU.mult,
                                    )
                                    wsel_ap = wsel[:]
                                nwi = ti if is_self else (
                                    S * WPT if is_ovf else ti * WPT + wi
                                )
                                for fh in range(nfh):
                                    r = ti * nfh + fh
                                    first = r % 4 == 0
                                    last = r % 4 == 3 or r == S * nfh - 1
                                    nc.tensor.matmul(
                                        out=selbk[r // 4][
                                            :, (r % 4) * P : (r % 4 + 1) * P
                                        ],
                                        lhsT=gv[:, nwi, fh * P : (fh + 1) * P],
                                        rhs=wsel_ap,
                                        start=bool(
                                            k == 0 and wi == 0 and first
                                        ),
                                        stop=bool(is_self and last),
                                    )
                    # ---- finish tiles of this sg ----
                    for ti in range(S):
                        t = tglob + ti
                        if lidx == 0:
                            s1 = sp.tile([P, P], BF16, tag="scopy")
                            nc.scalar.activation(
                                s1[:],
                                selbk[ti // 4][:, (ti % 4) * P : (ti % 4 + 1) * P],
                                AF.Copy,
                            )
                            hps = php.tile([P, DH], F32, space="PSUM", tag="hps")
                            nc.tensor.matmul(
                                out=hps[:], lhsT=s1[:], rhs=W1_t[:],
                                start=True, stop=False,
                            )
                            nc.tensor.matmul(
                                out=hps[:], lhsT=ones_t[:1, :], rhs=b1_t[:1, :],
                                start=False, stop=True,
                            )
                            h1sb = sp.tile([P, DH], F8, tag="hsb8")
                            nc.scalar.activation(h1sb[:], hps[:], AF.Relu)
                            nc.sync.dma_start(
                                h1shard[t * P : (t + 1) * P, :], h1sb[:]
                            )
                        else:
                            rl, rh = ti * 2, ti * 2 + 1
                            s2l = sp.tile([P, P], BF16, tag="scopy")
                            nc.scalar.activation(
                                s2l[:],
                                selbk[rl // 4][:, (rl % 4) * P : (rl % 4 + 1) * P],
                                AF.Copy,
                            )
                            s2h = sp.tile([P, P], BF16, tag="scopy2")
                            nc.scalar.activation(
                                s2h[:],
                                selbk[rh // 4][:, (rh % 4) * P : (rh % 4 + 1) * P],
                                AF.Copy,
                            )
                            hps = php.tile([P, DH], F32, space="PSUM", tag="hps")
                            nc.tensor.matmul(
                                out=hps[:], lhsT=s2l[:], rhs=W2lo_t[:],
                                start=True, stop=False,
                            )
                            nc.tensor.matmul(
                                out=hps[:], lhsT=s2h[:], rhs=W2hi_t[:],
                                start=False, stop=True,
                            )
                            # mean-pool 1/cnt folded into the PSUM->SBUF copy
                            h2sb = sp.tile([P, DH], BF16, tag="hsb")
                            nc.scalar.activation(
                                h2sb[:], hps[:], AF.Copy,
                                scale=bi_t[:, t : t + 1],
                            )
                            bsel = bp.tile([P, G], BF16, tag="bsel")
                            nc.vector.tensor_scalar(
                                out=bsel[:],
                                in0=io512[:],
                                scalar1=bl_t[:, t : t + 1],
                                scalar2=None,
                                op0=ALU.is_equal,
                            )
                            nc.tensor.matmul(
                                out=pl_lo[:], lhsT=h2sb[:, :P], rhs=bsel[:],
                                start=(t == 0), stop=(t == TPC - 1),
                            )
                            nc.tensor.matmul(
                                out=pl_hi[:], lhsT=h2sb[:, P:], rhs=bsel[:],
                                start=(t == 0), stop=(t == TPC - 1),
                            )
                    tglob += S
                    cbase += S * (NCHUNK * WPT + 1) + NCHUNK
                if lidx == 1:
                    return pl_lo, pl_hi

            layer(0)

            # ---- exchange h1 shards ----
            nc.gpsimd.collective_compute(
                "AllGather",
                ALU.bypass,
                replica_groups=[list(range(NCORES))],
                ins=[h1shard[:].opt()],
                outs=[h1full[:].opt()],
            )

            pl_lo, pl_hi = layer(1)

            # ---- pooled sums -> AllReduce (bf16: pooled means are O(1),
            # so bf16 keeps ~0.4% noise, far inside the tolerance) ----
            pb_in = dp.tile([DH, G], BF16)
            pb_out = dp.tile([DH, G], BF16)
            psl = sp.tile([P, G], BF16, tag="poolsb")
            nc.scalar.activation(psl[:], pl_lo[:], AF.Copy)
            psh = sp.tile([P, G], BF16, tag="poolsb2")
            nc.scalar.activation(psh[:], pl_hi[:], AF.Copy)
            nc.sync.dma_start(pb_in[:P, :], psl[:])
            nc.sync.dma_start(pb_in[P:, :], psh[:])
            nc.gpsimd.collective_compute(
                "AllReduce",
                ALU.add,
                replica_groups=[list(range(NCORES))],
                ins=[pb_in[:].opt()],
                outs=[pb_out[:].opt()],
            )

            # ---- MLP (replicated) ----
            plo_b = sp.tile([P, G], BF16, tag="poolbf")
            nc.sync.dma_start(plo_b[:], pb_out[:P, :])
            phi_b = sp.tile([P, G], BF16, tag="poolbf2")
            nc.sync.dma_start(phi_b[:], pb_out[P:, :])
            Wm1lo_t = cp.tile([P, DMLP], BF16)
            nc.sync.dma_start(Wm1lo_t[:], Wm1lo[:, :])
            Wm1hi_t = cp.tile([P, DMLP], BF16)
            nc.sync.dma_start(Wm1hi_t[:], Wm1hi[:, :])
            bm1_t = cp.tile([P, 4], F32)
            nc.sync.dma_start(bm1_t[:], bm1[:, :])
            Wm2_t = cp.tile([P, 4], BF16)
            nc.sync.dma_start(Wm2_t[:], Wm2[:, :])
            bm2_t = cp.tile([P, 1], F32)
            nc.sync.dma_start(bm2_t[:], bm2c[:, :])

            z2ps = php.tile([P, G], F32, space="PSUM", tag="hps")
            for osl in range(4):
                z1ps = pselp.tile([P, G], F32, space="PSUM", tag="selp")
                nc.tensor.matmul(
                    out=z1ps[:],
                    lhsT=Wm1lo_t[:, osl * P : (osl + 1) * P],
                    rhs=plo_b[:],
                    start=True, stop=False,
                )
                nc.tensor.matmul(
                    out=z1ps[:],
                    lhsT=Wm1hi_t[:, osl * P : (osl + 1) * P],
                    rhs=phi_b[:],
                    start=False, stop=True,
                )
                z1sb = sp.tile([P, G], BF16, tag="z1sb")
                nc.scalar.activation(
                    z1sb[:], z1ps[:], AF.Relu, bias=bm1_t[:, osl : osl + 1]
                )
                nc.tensor.matmul(
                    out=z2ps[:1, :],
                    lhsT=Wm2_t[:, osl : osl + 1],
                    rhs=z1sb[:],
                    start=(osl == 0), stop=(osl == 3),
                )
            osb = sp.tile([P, G], F32, tag="osb")
            nc.scalar.activation(osb[:1, :], z2ps[:1, :], AF.Sigmoid, bias=bm2_t[:1, :1])
            nc.sync.dma_start(out[:, :], osb[:1, :])

    nc.compile()
    return nc


LAST_EXEC_NS = None


def kernel(X, edge_index, batch, W1, b1, W2, b2, Wm1, bm1, Wm2, bm2, **_):
    global LAST_EXEC_NS
    in_maps, C, WPT, NW, mm_start, mm_stop, bm2_f = _prep(
        X, edge_index, batch, W1, b1, W2, b2, Wm1, bm1, Wm2, bm2
    )
    nc = _build(C, WPT, NW, mm_start, mm_stop, bm2_f)
    trace = os.environ.get("GCN_TRACE") == "1"
    try:
        res = run_bass_kernel_spmd(
            nc, in_maps, core_ids=list(range(NCORES)), trace=trace
        )
    except ModuleNotFoundError:
        res = run_bass_kernel_spmd(nc, in_maps, core_ids=list(range(NCORES)))
    LAST_EXEC_NS = res.exec_time_ns
    return np.asarray(res.results[0]["out"], np.float32).reshape(G, 1)



# revision 4
# speedup vs baseline: 1.4758x; 1.4758x over previous
"""GCN (2-layer + mean-pool + MLP) on 8 Trainium2 NeuronCores.

Strategy (dst-sharded message passing, matmul-based segment sum):
  - Nodes are tiled into 128-row tiles; each core owns 98 tiles (12544 nodes).
  - Edges are bucketed by (dst tile, src chunk) with a fixed per-bucket
    capacity C=256 (two 128-edge windows); bucket overflow beyond C spills
    into one shared overflow window per (supergroup, chunk), whose wsel
    passes target each tile of the supergroup in turn.  This keeps the
    instruction stream identical on every core (SPMD) while cutting gather
    descriptors ~25% vs padding every bucket to the max count.  Self loops
    get a dedicated window per tile whose source rows are contiguous in the
    local shard, so they are fetched with plain strided DMA, not dma_gather.
  - Aggregation S^T[f, d] = sum_e w_e * X[src_e, f] is computed on the PE:
    per 128-edge window, a weighted selection matrix Wsel[e, d] is built on
    the vector engine (iota==dstloc)*w via one dual-op tensor_scalar, and
    matmul(lhsT=G_window[e, f], rhs=Wsel) accumulates into the dst tile's
    PSUM region.  G windows are fetched with dma_gather (int16 indices,
    4 source chunks of 25088 rows each to satisfy the int16 range).
  - norm w_e = deg^-1/2[src] * deg^-1/2[dst] is folded into Wsel.
  - h1 is stored and exchanged in fp8e4m3 (halves the AllGather, the
    dominant serial cost; the quantization noise washes out in the
    mean-pool).  Layer 2 gathers fp8 rows and runs its window matmuls in
    fp8 x fp8 -> f32 PSUM.
  - PSUM->SBUF copies run on the Activation engine (Copy/Relu activations)
    to keep the vector engine free for Wsel builds; the mean-pool 1/cnt is
    folded into the h2 PSUM->SBUF copy as a per-partition activation scale.
  - Per-graph mean-pooling is a second selection matmul (Bsel =
    (iota512==batch)) accumulated over all tiles; pooled sums are AllReduced
    and the small MLP runs replicated.
"""

import os
import numpy as np
import ml_dtypes

from concourse import bass, mybir, tile, bacc
from concourse.bass_utils import run_bass_kernel_spmd

BF16 = mybir.dt.bfloat16
F32 = mybir.dt.float32
F8 = mybir.dt.float8e4
I16 = mybir.dt.int16
AF = mybir.ActivationFunctionType
ALU = mybir.AluOpType
NPF8 = mybir.dt.np(F8)

# problem sizes (hardcoded; see module docstring)
N, E, G = 100000, 800000, 512
DIN, DH, DMLP = 128, 256, 512
NCORES = 8
P = 128
TPC = 98                     # tiles per core (uniform; tail cores padded)
NT = NCORES * TPC            # 784 virtual tiles
NROWS = NT * P               # 100352 table rows (row == node id)
NCHUNK = 4
CHUNK = 25088                # source chunk rows (< 32768 for int16 idx)
SGS = [4] * 24 + [2]         # tiles per super-group, sum == TPC
SG_OF = []                   # tile-in-core -> (sg index, pos in sg)
for _si, _s in enumerate(SGS):
    for _j in range(_s):
        SG_OF.append((_si, _j))


def _prep(X, edge_index, batch, W1, b1, W2, b2, Wm1, bm1, Wm2, bm2):
    # self loops are handled by a dedicated per-sg bucket loaded contiguously
    # from the core-local shard (Xself / h1shard), so the random buckets stay
    # small.  Two edge streams per core: layer 1 gathers X, layer 2 gathers
    # the AllGathered h1 table (both in global row order).
    src = np.asarray(edge_index[0]).astype(np.int64)
    dst = np.asarray(edge_index[1]).astype(np.int64)
    batch = np.asarray(batch).astype(np.int64)
    SHARD = TPC * P

    deg = np.bincount(dst, minlength=N).astype(np.float32) + 1.0  # + self loop
    p = deg ** -0.5
    w = (p[src] * p[dst]).astype(np.float32)

    tile_id = (dst >> 7).astype(np.int64)        # 0..781
    core_of = tile_id // TPC

    # per-layer chunk ids and int16 index values
    chunk0 = src // CHUNK
    idxv0 = (src - chunk0 * CHUNK).astype(np.int16)
    rel_row = src                                # global row order (AllGather)
    chunk1 = rel_row // CHUNK
    idxv1 = (rel_row - chunk1 * CHUNK).astype(np.int16)

    C = 256                                      # per-tile bucket capacity
    OVF = 128                                    # shared overflow per (sg,k)
    WPT = C // P

    t_in_core = tile_id % TPC
    sg_idx_l = np.array([SG_OF[t][0] for t in range(TPC)])
    ti_sg_l = np.array([SG_OF[t][1] for t in range(TPC)])
    sg_sizes = np.array(SGS)
    # gather slots per sg: NCHUNK * (S*C + OVF); self loaded separately
    sg_slot_span = sg_sizes * NCHUNK * C + NCHUNK * OVF
    sg_slot_off = np.zeros(len(SGS) + 1, np.int64)
    np.cumsum(sg_slot_span, out=sg_slot_off[1:])
    SLOTS = int(sg_slot_off[-1])                 # gather slots per core
    # dl/w columns per sg: NCHUNK*(S*WPT regular + 1 ovf) + S self
    sg_col_span = sg_sizes * (NCHUNK * WPT + 1) + NCHUNK
    sg_col_off = np.zeros(len(SGS) + 1, np.int64)
    np.cumsum(sg_col_span, out=sg_col_off[1:])
    NW = int(sg_col_off[-1])                     # wsel columns per core

    sgi = sg_idx_l[t_in_core]
    tis = ti_sg_l[t_in_core]
    Ssz = sg_sizes[sgi]                          # sg size per edge

    def build_stream(chunk_id, idxv):
        key = tile_id * NCHUNK + chunk_id
        cnt = np.bincount(key, minlength=NT * NCHUNK)
        order = np.argsort(key, kind="stable")
        k_sorted = key[order]
        starts = np.zeros(NT * NCHUNK + 1, np.int64)
        np.cumsum(cnt, out=starts[1:])
        rank = np.empty(len(order), np.int64)
        rank[order] = np.arange(len(order)) - starts[k_sorted]

        reg = rank < C
        # overflow rank within (core, sg, chunk)
        okey = (core_of * len(SGS) + sgi) * NCHUNK + chunk_id
        osel = ~reg
        oorder = np.argsort(okey[osel], kind="stable")
        ocnt = np.bincount(okey[osel], minlength=NCORES * len(SGS) * NCHUNK)
        assert ocnt.max() <= OVF, f"overflow {ocnt.max()} > {OVF}"
        ostarts = np.zeros(NCORES * len(SGS) * NCHUNK + 1, np.int64)
        np.cumsum(ocnt, out=ostarts[1:])
        ovr = np.empty(osel.sum(), np.int64)
        ovr[oorder] = np.arange(osel.sum()) - ostarts[np.sort(okey[osel])]
        ovf_rank = np.zeros(len(src), np.int64)
        ovf_rank[osel] = ovr

        # gather slot of each edge
        kbase = sg_slot_off[sgi] + chunk_id * (Ssz * C + OVF)
        slot = np.where(
            reg,
            kbase + tis * C + rank,
            kbase + Ssz * C + ovf_rank,
        )
        # wsel column of each edge (one shared ovf column per (sg,k))
        cbase = sg_col_off[sgi] + chunk_id * (Ssz * WPT + 1)
        colid = np.where(
            reg,
            cbase + tis * WPT + rank // P,
            cbase + Ssz * WPT,
        )
        # position within the window
        pwin = np.where(reg, rank % P, ovf_rank)

        idx_flat = np.zeros((NCORES, SLOTS), np.int16)
        dl_flat = np.full((NCORES, NW, P), -1.0, np.float32)
        w_flat = np.zeros((NCORES, NW, P), np.float32)
        idx_flat[core_of, slot] = idxv
        sg_t0 = np.concatenate([[0], np.cumsum(sg_sizes)])[sgi]
        sg_base = (core_of * TPC + sg_t0) * P
        dl_flat[core_of, colid, pwin] = np.where(
            reg, dst - tile_id * P, dst - sg_base
        ).astype(np.float32)
        w_flat[core_of, colid, pwin] = w
        # self-loop entries: node n (real) -> self column of its sg
        nodes = np.arange(N)
        tl = nodes >> 7
        tc = tl % TPC
        co = tl // TPC
        sgi_n = sg_idx_l[tc]
        scol = (
            sg_col_off[sgi_n]
            + NCHUNK * (sg_sizes[sgi_n] * WPT + 1)
            + ti_sg_l[tc]
        )
        dl_flat[co, scol, nodes % P] = (nodes % P).astype(np.float32)
        w_flat[co, scol, nodes % P] = (p[nodes] ** 2).astype(np.float32)
        return idx_flat, dl_flat, w_flat

    idx_flat0, dl_flat0, w_flat0 = build_stream(chunk0, idxv0)
    idx_flat1, dl_flat1, w_flat1 = build_stream(chunk1, idxv1)

    mm_start = mm_stop = None                    # flags derived inline

    in_maps = []
    tbl = np.zeros((NROWS, DIN), ml_dtypes.bfloat16)
    tbl[:N] = np.asarray(X).astype(ml_dtypes.bfloat16)

    cnts_g = np.bincount(batch, minlength=G).astype(np.float32)
    invc = (1.0 / np.maximum(cnts_g, 1.0)).astype(np.float32)

    common = dict(
        tbl=tbl,
        iota128=np.tile(np.arange(P, dtype=ml_dtypes.bfloat16)[None, :], (P, 1)),
        iota512=np.tile(np.arange(G, dtype=np.float32)[None, :], (P, 1)),
        W1=np.asarray(W1).astype(ml_dtypes.bfloat16),
        b1=np.asarray(b1).astype(ml_dtypes.bfloat16)[None, :],
        W2lo=np.asarray(W2[:128]).astype(ml_dtypes.bfloat16),
        W2hi=np.asarray(W2[128:]).astype(ml_dtypes.bfloat16),
        b2=np.asarray(b2).astype(ml_dtypes.bfloat16)[None, :],
        Wm1lo=np.asarray(Wm1[:128]).astype(ml_dtypes.bfloat16),
        Wm1hi=np.asarray(Wm1[128:]).astype(ml_dtypes.bfloat16),
        # layer-2 bias folded through the linear mean-pool: bm1 += b2 @ Wm1
        bm1=(
            np.asarray(bm1).astype(np.float32)
            + np.asarray(b2).astype(np.float32) @ np.asarray(Wm1).astype(np.float32)
        ).reshape(4, 128).T.copy(),
        Wm2=np.asarray(Wm2).astype(ml_dtypes.bfloat16).reshape(4, 128).T.copy(),
    )
    bm2_f = float(np.asarray(bm2).reshape(-1)[0])
    common["bm2c"] = np.full((P, 1), bm2_f, np.float32)

    IDXBLOB = SLOTS                          # gather slots per core

    def wrap_idx(idx_flat, c):
        idx_c = idx_flat[c]
        # per-(sg,k) [16, L/16] wrap, replicated to 128 partitions
        idxw = np.empty((16, IDXBLOB // 16), np.int16)
        off = 0
        for si, S in enumerate(SGS):
            for L in [S * C + OVF] * NCHUNK:
                a = idx_c[off : off + L].reshape(L // 16, 16).T
                idxw[:, off // 16 : (off + L) // 16] = a
                off += L
        return np.tile(idxw, (8, 1))

    for c in range(NCORES):
        idxw0 = wrap_idx(idx_flat0, c)
        idxw1 = wrap_idx(idx_flat1, c)
        xself = np.zeros((TPC * P, DIN), ml_dtypes.bfloat16)
        lo, hi = c * TPC * P, min((c + 1) * TPC * P, N)
        xself[: hi - lo] = np.asarray(X[lo:hi]).astype(ml_dtypes.bfloat16)

        dl0_c = dl_flat0[c].T.copy()
        w0_c = w_flat0[c].T.copy()
        dl1_c = dl_flat1[c].T.copy()
        w1_c = w_flat1[c].T.copy()

        # pooling metadata per tile
        n0 = c * TPC * P
        nodes = np.arange(n0, n0 + TPC * P)
        valid = nodes < N
        bl = np.where(valid, batch[np.minimum(nodes, N - 1)], -1).astype(np.float32)
        iv = np.where(valid, invc[batch[np.minimum(nodes, N - 1)]], 0.0).astype(
            np.float32
        )
        in_maps.append(
            dict(
                common,
                idx0=idxw0,
                idx1=idxw1,
                dloc0=dl0_c,
                wgt0=w0_c,
                dloc1=dl1_c,
                wgt1=w1_c,
                xself=xself,
                bloc=bl.reshape(TPC, P).T.copy(),
                binv=iv.reshape(TPC, P).T.copy(),
            )
        )
    return in_maps, C, WPT, NW, mm_start, mm_stop, bm2_f


_GQ = [0]


def _next_gq():
    q = _GQ[0]
    _GQ[0] = (q + 1) % 4
    return q


def _build(C, WPT, NW, mm_start, mm_stop, bm2_f):
    _GQ[0] = 0
    nc = bacc.Bacc(
        "TRN2",
        target_bir_lowering=False,
        debug=False,
        num_devices=NCORES,
        dynamic_dma_scratch_size=32768,
        num_swdge_queues=4,
    )

    NSG = len(SGS)
    OVF = 128
    IDXBLOB = NCHUNK * (C * TPC + OVF * NSG)
    tbl = nc.dram_tensor("tbl", [NROWS, DIN], BF16, kind="ExternalInput")
    xself = nc.dram_tensor("xself", [TPC * P, DIN], BF16, kind="ExternalInput")
    idx0 = nc.dram_tensor("idx0", [P, IDXBLOB // 16], I16, kind="ExternalInput")
    idx1 = nc.dram_tensor("idx1", [P, IDXBLOB // 16], I16, kind="ExternalInput")
    dloc0 = nc.dram_tensor("dloc0", [P, NW], F32, kind="ExternalInput")
    wgt0 = nc.dram_tensor("wgt0", [P, NW], F32, kind="ExternalInput")
    dloc1 = nc.dram_tensor("dloc1", [P, NW], F32, kind="ExternalInput")
    wgt1 = nc.dram_tensor("wgt1", [P, NW], F32, kind="ExternalInput")
    bloc = nc.dram_tensor("bloc", [P, TPC], F32, kind="ExternalInput")
    binv = nc.dram_tensor("binv", [P, TPC], F32, kind="ExternalInput")
    iota128 = nc.dram_tensor("iota128", [P, P], BF16, kind="ExternalInput")
    iota512 = nc.dram_tensor("iota512", [P, G], F32, kind="ExternalInput")
    W1 = nc.dram_tensor("W1", [DIN, DH], BF16, kind="ExternalInput")
    b1 = nc.dram_tensor("b1", [1, DH], BF16, kind="ExternalInput")
    W2lo = nc.dram_tensor("W2lo", [P, DH], BF16, kind="ExternalInput")
    W2hi = nc.dram_tensor("W2hi", [P, DH], BF16, kind="ExternalInput")
    b2 = nc.dram_tensor("b2", [1, DH], BF16, kind="ExternalInput")
    Wm1lo = nc.dram_tensor("Wm1lo", [P, DMLP], BF16, kind="ExternalInput")
    Wm1hi = nc.dram_tensor("Wm1hi", [P, DMLP], BF16, kind="ExternalInput")
    bm1 = nc.dram_tensor("bm1", [P, 4], F32, kind="ExternalInput")
    Wm2 = nc.dram_tensor("Wm2", [P, 4], BF16, kind="ExternalInput")
    bm2c = nc.dram_tensor("bm2c", [P, 1], F32, kind="ExternalInput")
    out = nc.dram_tensor("out", [1, G], F32, kind="ExternalOutput")

    NIDX16 = IDXBLOB // 16
    GC1 = (4 * 2 + 1) * DIN                      # L1 gather buffer elems
    GC2 = (4 * 2 + 1) * DH                       # L2 gather buffer elems

    with tile.TileContext(nc) as tc:
        with (
            tc.tile_pool(name="const", bufs=1) as cp,
            tc.tile_pool(name="gbuf", bufs=10) as gp,
            tc.tile_pool(name="wselp", bufs=24) as wp,
            tc.tile_pool(name="stage", bufs=6) as sp,
            tc.tile_pool(name="bselp", bufs=8) as bp,
            tc.tile_pool(name="selpsum", bufs=4, space="PSUM") as pselp,
            tc.tile_pool(name="hpsum", bufs=2, space="PSUM") as php,
            tc.tile_pool(name="poolpsum", bufs=1, space="PSUM") as ppl,
            tc.tile_pool(name="dram", bufs=1, space="DRAM") as dp,
        ):
            # ---- resident constants / metadata ----
            # one idx tile, reloaded with the layer-2 stream between layers
            idx_t = cp.tile([P, NIDX16], I16)
            nc.sync.dma_start(idx_t[:], idx0[:, :])
            dl_ts = [cp.tile([P, NW], F32, name=f"dl{l}") for l in range(2)]
            w_ts = [cp.tile([P, NW], F32, name=f"w{l}") for l in range(2)]
            nc.sync.dma_start(dl_ts[0][:], dloc0[:, :])
            nc.sync.dma_start(w_ts[0][:], wgt0[:, :])
            bl_t = cp.tile([P, TPC], F32)
            nc.sync.dma_start(bl_t[:], bloc[:, :])
            bi_t = cp.tile([P, TPC], F32)
            nc.sync.dma_start(bi_t[:], binv[:, :])
            io128 = cp.tile([P, P], BF16)
            nc.sync.dma_start(io128[:], iota128[:, :])
            io512 = cp.tile([P, G], F32)
            nc.sync.dma_start(io512[:], iota512[:, :])
            W1_t = cp.tile([DIN, DH], BF16)
            nc.sync.dma_start(W1_t[:], W1[:, :])
            b1_t = cp.tile([P, DH], BF16)
            nc.sync.dma_start(b1_t[:1, :], b1[:, :])
            W2lo_t = cp.tile([P, DH], BF16)
            nc.sync.dma_start(W2lo_t[:], W2lo[:, :])
            W2hi_t = cp.tile([P, DH], BF16)
            nc.sync.dma_start(W2hi_t[:], W2hi[:, :])
            b2_t = cp.tile([P, DH], BF16)
            nc.sync.dma_start(b2_t[:1, :], b2[:, :])
            ones_t = cp.tile([P, P], BF16)
            nc.vector.memset(ones_t[:], 1.0)

            h1shard = dp.tile([TPC * P, DH], F8)
            # Shared addr space: HBM-HBM AllGather writes peer-visible
            # memory directly (the runtime warns Local is slower).
            h1full = nc.dram_tensor(
                "h1full_sh", [NROWS, DH], F8, kind="Internal",
                addr_space="Shared",
            )
            # ================= layer phase =================
            def layer(lidx):
                """lidx 0: gather X(128) -> h1; lidx 1: gather h1(256) -> h2+pool."""
                feat = DIN if lidx == 0 else DH
                gdt = BF16 if lidx == 0 else F8
                nfh = feat // P                      # feature halves
                src_tbl = tbl if lidx == 0 else h1full
                self_tbl = xself if lidx == 0 else h1shard
                dl_t = dl_ts[lidx]
                w_t = w_ts[lidx]
                tglob = 0
                off16 = 0
                cbase = 0                        # column base of current sg
                if lidx == 1:
                    # swap in the layer-2 gather indices / edge metadata
                    nc.sync.dma_start(idx_t[:], idx1[:, :])
                    nc.sync.dma_start(dl_ts[1][:], dloc1[:, :])
                    nc.sync.dma_start(w_ts[1][:], wgt1[:, :])
                    pl_lo = ppl.tile([P, G], F32, space="PSUM")
                    pl_hi = ppl.tile([P, G], F32, space="PSUM")
                for si, S in enumerate(SGS):
                    # PSUM for this sg: one accumulation region [P, P] per
                    # (tile, fhalf); regions packed 4-per-bank, one pool tile
                    # per 2KB bank (start=True zeroes a whole bank).
                    nbank = (S * nfh + 3) // 4
                    selbk = [
                        pselp.tile(
                            [P, 512], F32, space="PSUM", tag="selp",
                            name=f"selbk_{lidx}_{si}_{b}",
                        )
                        for b in range(nbank)
                    ]
                    for k in range(NCHUNK + 1):      # 4 chunk buckets + self
                        is_self = k == NCHUNK
                        L = S * P if is_self else S * C + OVF
                        nwin = L // P
                        g_t = gp.tile(
                            [P, GC1 if lidx == 0 else GC2], gdt, tag=f"gbuf{lidx}"
                        )
                        gv = g_t[:, : nwin * feat].rearrange(
                            "p (n f) -> p n f", f=feat
                        )
                        if is_self:
                            # self rows are contiguous in the local shard:
                            # plain strided DMA instead of dma_gather.  L1
                            # issues them from the scalar queue to keep SP
                            # free for const loads / h1 writes.
                            for ti in range(S):
                                t = tglob + ti
                                eng = (
                                    (nc.scalar if t % 2 else nc.sync)
                                    if lidx == 0
                                    else nc.sync
                                )
                                eng.dma_start(
                                    gv[:, ti, :],
                                    self_tbl[t * P : (t + 1) * P, :],
                                )
                        else:
                            # <=8 windows (1024 idxs) per dma_gather call:
                            # larger calls exceed the SWDGE ring sizing the
                            # terminal ucode assumes and crash the worker.
                            src_ap = src_tbl[k * CHUNK : (k + 1) * CHUNK, :]
                            for g0 in range(0, nwin, 8):
                                gn = min(8, nwin - g0)
                                nc.gpsimd.dma_gather(
                                    out_ap=gv[:, g0 : g0 + gn, :],
                                    in_ap=src_ap,
                                    idxs_ap=idx_t[
                                        :, off16 + g0 * 8 : off16 + (g0 + gn) * 8
                                    ],
                                    num_idxs=gn * P,
                                    num_idxs_reg=gn * P,
                                    elem_size=feat,
                                    queue_num=_next_gq(),
                                )
                            off16 += L // 16
                        # window index per (tile, wi): regular 2 per tile,
                        # plus one shared overflow window (last) per chunk.
                        # The ovf wsel is built ONCE per (sg,k) spanning the
                        # whole sg (dloc = dst - sg_base); each tile's matmul
                        # consumes its 128-column slice, which is exactly the
                        # per-tile one-hot.
                        wpb = 1 if is_self else WPT + 1
                        for ti in range(S):
                            for wi in range(wpb):
                                is_ovf = (not is_self) and wi == WPT
                                if is_self:
                                    ccol = (
                                        cbase + NCHUNK * (S * WPT + 1) + ti
                                    )
                                elif is_ovf:
                                    ccol = (
                                        cbase + k * (S * WPT + 1) + S * WPT
                                    )
                                else:
                                    ccol = (
                                        cbase + k * (S * WPT + 1)
                                        + ti * WPT + wi
                                    )
                                if is_ovf:
                                    if ti == 0:
                                        wselo = wp.tile(
                                            [P, 4 * P], gdt,
                                            tag=f"wselo{lidx}",
                                        )
                                        nc.vector.tensor_scalar(
                                            out=wselo[:, : S * P],
                                            in0=io512[:, : S * P],
                                            scalar1=dl_t[:, ccol : ccol + 1],
                                            scalar2=w_t[:, ccol : ccol + 1],
                                            op0=ALU.is_equal,
                                            op1=ALU.mult,
                                        )
                                    wsel_ap = wselo[:, ti * P : (ti + 1) * P]
                                else:
                                    wsel = wp.tile(
                                        [P, P], gdt, tag=f"wsel{lidx}"
                                    )
                                    # L1 self wsels alternate onto gpsimd:
                                    # Pool has slack in L1 while DVE binds.
                                    weng = (
                                        nc.gpsimd
                                        if is_self and lidx == 0
                                        else nc.vector
                                    )
                                    weng.tensor_scalar(
                                        out=wsel[:],
                                        in0=io128[:],
                                        scalar1=dl_t[:, ccol : ccol + 1],
                                        scalar2=w_t[:, ccol : ccol + 1],
                                        op0=ALU.is_equal,
                                        op1=ALU.mult,
                                    )
                                    wsel_ap = wsel[:]
                                nwi = ti if is_self else (
                                    S * WPT if is_ovf else ti * WPT + wi
                                )
                                for fh in range(nfh):
                                    r = ti * nfh + fh
                                    first = r % 4 == 0
                                    last = r % 4 == 3 or r == S * nfh - 1
                                    nc.tensor.matmul(
                                        out=selbk[r // 4][
                                            :, (r % 4) * P : (r % 4 + 1) * P
                                        ],
                                        lhsT=gv[:, nwi, fh * P : (fh + 1) * P],
                                        rhs=wsel_ap,
                                        start=bool(
                                            k == 0 and wi == 0 and first
                                        ),
                                        stop=bool(is_self and last),
                                    )
                    # ---- finish tiles of this sg ----
                    for ti in range(S):
                        t = tglob + ti
                        if lidx == 0:
                            s1 = sp.tile([P, P], BF16, tag="scopy")
                            nc.scalar.activation(
                                s1[:],
                                selbk[ti // 4][:, (ti % 4) * P : (ti % 4 + 1) * P],
                                AF.Copy,
                            )
                            hps = php.tile([P, DH], F32, space="PSUM", tag="hps")
                            nc.tensor.matmul(
                                out=hps[:], lhsT=s1[:], rhs=W1_t[:],
                                start=True, stop=False,
                            )
                            nc.tensor.matmul(
                                out=hps[:], lhsT=ones_t[:1, :], rhs=b1_t[:1, :],
                                start=False, stop=True,
                            )
                            h1sb = sp.tile([P, DH], F8, tag="hsb8")
                            nc.scalar.activation(h1sb[:], hps[:], AF.Relu)
                            nc.sync.dma_start(
                                h1shard[t * P : (t + 1) * P, :], h1sb[:]
                            )
                        else:
                            rl, rh = ti * 2, ti * 2 + 1
                            s2l = sp.tile([P, P], BF16, tag="scopy")
                            nc.scalar.activation(
                                s2l[:],
                                selbk[rl // 4][:, (rl % 4) * P : (rl % 4 + 1) * P],
                                AF.Copy,
                            )
                            s2h = sp.tile([P, P], BF16, tag="scopy2")
                            nc.scalar.activation(
                                s2h[:],
                                selbk[rh // 4][:, (rh % 4) * P : (rh % 4 + 1) * P],
                                AF.Copy,
                            )
                            hps = php.tile([P, DH], F32, space="PSUM", tag="hps")
                            nc.tensor.matmul(
                                out=hps[:], lhsT=s2l[:], rhs=W2lo_t[:],
                                start=True, stop=False,
                            )
                            nc.tensor.matmul(
                                out=hps[:], lhsT=s2h[:], rhs=W2hi_t[:],
                                start=False, stop=True,
                            )
                            # mean-pool 1/cnt folded into the PSUM->SBUF copy
                            h2sb = sp.tile([P, DH], BF16, tag="hsb")
                            nc.scalar.activation(
                                h2sb[:], hps[:], AF.Copy,
                                scale=bi_t[:, t : t + 1],
                            )
                            bsel = bp.tile([P, G], BF16, tag="bsel")
                            nc.vector.tensor_scalar(
                                out=bsel[:],
                                in0=io512[:],
                                scalar1=bl_t[:, t : t + 1],
                                scalar2=None,
                                op0=ALU.is_equal,
                            )
                            nc.tensor.matmul(
                                out=pl_lo[:], lhsT=h2sb[:, :P], rhs=bsel[:],
                                start=(t == 0), stop=(t == TPC - 1),
                            )
                            nc.tensor.matmul(
                                out=pl_hi[:], lhsT=h2sb[:, P:], rhs=bsel[:],
                                start=(t == 0), stop=(t == TPC - 1),
                            )
                    tglob += S
                    cbase += S * (NCHUNK * WPT + 1) + NCHUNK
                if lidx == 1:
                    return pl_lo, pl_hi

            layer(0)

            # ---- exchange h1 shards ----
            nc.gpsimd.collective_compute(
                "AllGather",
                ALU.bypass,
                replica_groups=[list(range(NCORES))],
                ins=[h1shard[:].opt()],
                outs=[h1full[:].opt()],
            )

            pl_lo, pl_hi = layer(1)

            # ---- pooled sums -> AllReduce (bf16: pooled means are O(1),
            # so bf16 keeps ~0.4% noise, far inside the tolerance) ----
            pb_in = dp.tile([DH, G], BF16)
            pb_out = dp.tile([DH, G], BF16)
            psl = sp.tile([P, G], BF16, tag="poolsb")
            nc.scalar.activation(psl[:], pl_lo[:], AF.Copy)
            psh = sp.tile([P, G], BF16, tag="poolsb2")
            nc.scalar.activation(psh[:], pl_hi[:], AF.Copy)
            nc.sync.dma_start(pb_in[:P, :], psl[:])
            nc.sync.dma_start(pb_in[P:, :], psh[:])
            nc.gpsimd.collective_compute(
                "AllReduce",
                ALU.add,
                replica_groups=[list(range(NCORES))],
                ins=[pb_in[:].opt()],
                outs=[pb_out[:].opt()],
            )

            # ---- MLP (replicated) ----
            plo_b = sp.tile([P, G], BF16, tag="poolbf")
            nc.sync.dma_start(plo_b[:], pb_out[:P, :])
            phi_b = sp.tile([P, G], BF16, tag="poolbf2")
            nc.sync.dma_start(phi_b[:], pb_out[P:, :])
            Wm1lo_t = cp.tile([P, DMLP], BF16)
            nc.sync.dma_start(Wm1lo_t[:], Wm1lo[:, :])
            Wm1hi_t = cp.tile([P, DMLP], BF16)
            nc.sync.dma_start(Wm1hi_t[:], Wm1hi[:, :])
            bm1_t = cp.tile([P, 4], F32)
            nc.sync.dma_start(bm1_t[:], bm1[:, :])
            Wm2_t = cp.tile([P, 4], BF16)
            nc.sync.dma_start(Wm2_t[:], Wm2[:, :])
            bm2_t = cp.tile([P, 1], F32)
            nc.sync.dma_start(bm2_t[:], bm2c[:, :])

            z2ps = php.tile([P, G], F32, space="PSUM", tag="hps")
            for osl in range(4):
                z1ps = pselp.tile([P, G], F32, space="PSUM", tag="selp")
                nc.tensor.matmul(
                    out=z1ps[:],
                    lhsT=Wm1lo_t[:, osl * P : (osl + 1) * P],
                    rhs=plo_b[:],
                    start=True, stop=False,
                )
                nc.tensor.matmul(
                    out=z1ps[:],
                    lhsT=Wm1hi_t[:, osl * P : (osl + 1) * P],
                    rhs=phi_b[:],
                    start=False, stop=True,
                )
                z1sb = sp.tile([P, G], BF16, tag="z1sb")
                nc.scalar.activation(
                    z1sb[:], z1ps[:], AF.Relu, bias=bm1_t[:, osl : osl + 1]
                )
                nc.tensor.matmul(
                    out=z2ps[:1, :],
                    lhsT=Wm2_t[:, osl : osl + 1],
                    rhs=z1sb[:],
                    start=(osl == 0), stop=(osl == 3),
                )
            osb = sp.tile([P, G], F32, tag="osb")
            nc.scalar.activation(osb[:1, :], z2ps[:1, :], AF.Sigmoid, bias=bm2_t[:1, :1])
            nc.sync.dma_start(out[:, :], osb[:1, :])

    nc.compile()
    return nc


LAST_EXEC_NS = None


def kernel(X, edge_index, batch, W1, b1, W2, b2, Wm1, bm1, Wm2, bm2, **_):
    global LAST_EXEC_NS
    in_maps, C, WPT, NW, mm_start, mm_stop, bm2_f = _prep(
        X, edge_index, batch, W1, b1, W2, b2, Wm1, bm1, Wm2, bm2
    )
    nc = _build(C, WPT, NW, mm_start, mm_stop, bm2_f)
    trace = os.environ.get("GCN_TRACE") == "1"
    try:
        res = run_bass_kernel_spmd(
            nc, in_maps, core_ids=list(range(NCORES)), trace=trace
        )
    except ModuleNotFoundError:
        res = run_bass_kernel_spmd(nc, in_maps, core_ids=list(range(NCORES)))
    LAST_EXEC_NS = res.exec_time_ns
    return np.asarray(res.results[0]["out"], np.float32).reshape(G, 1)

